# revision 15
# baseline (speedup 1.0000x reference)
"""DeepSeekMoE layer on 8 TRN2 NeuronCores — expert-parallel with host token dispatch.

Reference computation (per token):
    shared = silu(x @ ws1) @ ws2
    router: softmax(x @ w_router) -> top-2 -> renormalize -> gates
    routed = sum_{e in top2} gate_e * silu(x @ w1[e]) @ w2[e]
    out    = shared + routed

Sharding: expert-parallel. Core e receives the (padded) bucket of all token
rows routed to expert e (capacity C), plus a 1/8 slice of all tokens for the
shared expert. Routing (softmax/top-k) and the dispatch/combine permutations
run on the host; all GEMMs + SiLU + gate scaling run on device.

Per-core device kernel (two passes through a DRAM-resident intermediate):
    pass1: hT = silu(w1e.T @ xT)   [I x R] with I=1408 on partitions
    pass2: y  = hT.T @ w2e         [R x H] scaled per-row by the gate
All matmuls run as float32r (full fp32 storage; TensorE full-rate mode).
"""

import numpy as np

import concourse.bass as bass  # noqa: F401  (bass types via bacc/tile)
import concourse.mybir as mybir
import concourse.tile as tile
from concourse import bacc
from concourse.bass_utils import run_bass_kernel_spmd

H = 2048          # hidden
I = 1408          # moe intermediate
E = 8             # routed experts == n cores
NCORES = 8
RT = 512          # token tile (columns of xT / rows of y) per step
KH = H // 128     # 16 k-tiles over hidden
KI = I // 128     # 11 k-tiles over intermediate
F32 = mybir.dt.float32
F32R = mybir.dt.float32r

_BUILD_CACHE: dict = {}


def _r(ap):
    return ap


NPRE = 4          # weight k-slices prefetched into wpre for the next phase
STORE_ENGINE = "sync"   # which engine issues output stores: sync|gpsimd|scalar|vector
IN_BUFS = 4


def _store(nc, dst, src):
    getattr(nc, STORE_ENGINE).dma_start(dst, src)


def _silu_evict(nc, out_pool, ps, tag_id, use_silu, sz=RT):
    ev = out_pool.tile([128, RT], F32R, tag="ev", name=f"ev_{tag_id}")
    if use_silu:
        nc.scalar.activation(ev[:, :sz], ps[:], mybir.ActivationFunctionType.Silu)
    else:
        sg = out_pool.tile([128, RT], F32, tag="sg", bufs=2, name=f"sg_{tag_id}")
        nc.scalar.activation(sg[:, :sz], ps[:], mybir.ActivationFunctionType.Sigmoid)
        nc.vector.tensor_mul(ev[:, :sz], ps[:], sg[:, :sz])
    return ev


def _tiles(ncols):
    """R-tile (offset, size) list: 512-tiles plus an optional 256 tail."""
    out, off = [], 0
    while ncols - off >= RT:
        out.append((off, RT))
        off += RT
    if ncols - off:
        assert (ncols - off) % 256 == 0
        out.append((off, ncols - off))
        off = ncols
    return out


def _emit_pass1(nc, pools, wpool, w_dram, w_pre, xt, ht, ncols, use_silu, ph):
    """ht[:, i, :] = silu(w.T @ xT) — I on partitions, tokens on free.

    First R-tile: per-k sliced stream+weight DMAs in consumption order and a
    k-outer chunked matmul loop, so the PE consumes slices as they arrive.
    Later R-tiles: i-outer / k-inner with whole-half stream DMAs.
    """
    in_pool, out_pool, psum_pool = pools
    npre = len(w_pre) if w_pre else 0

    # --- first R-tile: sliced, streaming ---
    cs = slice(0, RT)
    xh0 = in_pool.tile([128, 8, RT], F32R, tag="sin", name=f"xh0_{ph}_0")
    xh1 = in_pool.tile([128, 8, RT], F32R, tag="sin", name=f"xh1_{ph}_0")
    w = wpool.tile([128, KH, I], F32R, tag="w", name=f"w_{ph}")

    def xslice(k):
        return (xh0 if k < 8 else xh1)[:, k % 8, :]

    if npre:
        # stream slices first (prefetchable), then the WAR-blocked weight rest
        for k in range(KH):
            nc.sync.dma_start(xslice(k), xt[:, k, cs])
        for k in range(npre, KH):
            nc.sync.dma_start(w[:, k, :], w_dram[:, k, :])
    else:
        # cold start: interleave in consumption order
        for k in range(KH):
            nc.sync.dma_start(xslice(k), xt[:, k, cs])
            nc.sync.dma_start(w[:, k, :], w_dram[:, k, :])

    for lo, hi in ((0, 6), (6, KI)):
        pss = [psum_pool.tile([128, RT], F32, tag="ps", name=f"ps_{ph}_0_{i}")
               for i in range(lo, hi)]
        for k in range(KH):
            w_k = w_pre[k] if k < npre else w[:, k, :]
            for i in range(lo, hi):
                nc.tensor.matmul(
                    pss[i - lo][:], w_k[:, i * 128:(i + 1) * 128], xslice(k),
                    start=(k == 0), stop=(k == KH - 1))
        for i in range(lo, hi):
            ev = _silu_evict(nc, out_pool, pss[i - lo], f"{ph}_0_{i}", use_silu)
            _store(nc, ht[:, i, cs], ev[:])
    if npre:
        # also load the wpre-covered slices into the main tile for later R-tiles
        for k in range(npre):
            nc.sync.dma_start(w[:, k, :], w_dram[:, k, :])

    next_pre = None

    # --- remaining R-tiles (maybe a 256-wide tail) ---
    for r, (off, sz) in enumerate(_tiles(ncols)):
        if r == 0:
            continue
        cs = slice(off, off + sz)
        xh0 = in_pool.tile([128, 8, sz], F32R, tag="sin", name=f"xh0_{ph}_{r}")
        xh1 = in_pool.tile([128, 8, sz], F32R, tag="sin", name=f"xh1_{ph}_{r}")
        nc.sync.dma_start(xh0[:], xt[:, 0:8, cs])
        nc.sync.dma_start(xh1[:], xt[:, 8:16, cs])
        for i in range(KI):
            ps = psum_pool.tile([128, sz], F32, tag="ps", name=f"ps_{ph}_{r}_{i}")
            for k in range(KH):
                nc.tensor.matmul(
                    ps[:], w[:, k, i * 128:(i + 1) * 128],
                    (xh0 if k < 8 else xh1)[:, k % 8, :],
                    start=(k == 0), stop=(k == KH - 1))
            ev = _silu_evict(nc, out_pool, ps, f"{ph}_{r}_{i}", use_silu, sz)
            _store(nc, ht[:, i, cs], ev[:, :sz])
        if r == 1:
            next_pre = yield  # build() emits the next phase's wpre DMAs here
    if len(_tiles(ncols)) < 2:
        next_pre = yield
    yield next_pre


def _emit_pass2(nc, pools, wpool, w_dram, w_pre, ht, y, ncols, scale_sb, ph):
    """y[m, :] = (hT.T @ w2) * gate[m] — tokens on partitions.

    First R-tile: per-i sliced DMAs + i-outer over two chunks of 8 psum banks.
    Later R-tiles: i-inner.
    """
    in_pool, out_pool, psum_pool = pools
    npre = len(w_pre) if w_pre else 0

    def evict(ps, m, hblk):
        yt = out_pool.tile([128, 512], F32, tag="ev", name=f"y_{ph}_{m}_{hblk}")
        if scale_sb is not None:
            nc.vector.tensor_scalar_mul(yt[:], ps[:], scale_sb[:, m:m + 1])
        else:
            nc.vector.tensor_copy(yt[:], ps[:])
        _store(nc, y[:, m, hblk * 512:(hblk + 1) * 512], yt[:])

    # --- first R-tile: sliced, streaming ---
    cs = slice(0, RT)
    hh0 = in_pool.tile([128, 6, RT], F32R, tag="sin", name=f"hh0_{ph}_0")
    hh1 = in_pool.tile([128, 5, RT], F32R, tag="sin", name=f"hh1_{ph}_0")
    w = wpool.tile([128, KI, H], F32R, tag="w", name=f"w_{ph}")

    def hslice(i):
        return (hh0 if i < 6 else hh1)[:, i if i < 6 else i - 6, :]

    def lhs(i, c):
        return hslice(i)[:, c * 128:(c + 1) * 128]

    for i in range(KI):
        nc.sync.dma_start(hslice(i), ht[:, i, cs])
    for i in range(npre, KI):
        nc.sync.dma_start(w[:, i, :], w_dram[:, i, :])

    pairs = [(c, hb) for c in range(RT // 128) for hb in range(H // 512)]
    for chunk in (pairs[:8], pairs[8:]):
        pss = {p: psum_pool.tile([128, 512], F32, tag="ps",
                                 name=f"ps_{ph}_0_{p[0]}_{p[1]}")
               for p in chunk}
        for i in range(KI):
            w_i = w_pre[i] if i < npre else w[:, i, :]
            for (c, hb) in chunk:
                nc.tensor.matmul(
                    pss[(c, hb)][:], lhs(i, c), w_i[:, hb * 512:(hb + 1) * 512],
                    start=(i == 0), stop=(i == KI - 1))
        for (c, hb) in chunk:
            evict(pss[(c, hb)], c, hb)
    for i in range(npre):
        nc.sync.dma_start(w[:, i, :], w_dram[:, i, :])

    next_pre = None

    # --- remaining R-tiles (maybe a 256-wide tail) ---
    for r, (off, sz) in enumerate(_tiles(ncols)):
        if r == 0:
            continue
        cs = slice(off, off + sz)
        hh0 = in_pool.tile([128, 6, sz], F32R, tag="sin", name=f"hh0_{ph}_{r}")
        hh1 = in_pool.tile([128, 5, sz], F32R, tag="sin", name=f"hh1_{ph}_{r}")
        nc.sync.dma_start(hh0[:], ht[:, 0:6, cs])
        nc.sync.dma_start(hh1[:], ht[:, 6:KI, cs])
        for c in range(sz // 128):
            for hb in range(H // 512):
                m = off // 128 + c
                ps = psum_pool.tile([128, 512], F32, tag="ps", name=f"ps_{ph}_{m}_{hb}")
                for i in range(KI):
                    src = hh0 if i < 6 else hh1
                    j = i if i < 6 else i - 6
                    nc.tensor.matmul(
                        ps[:], src[:, j, c * 128:(c + 1) * 128],
                        w[:, i, hb * 512:(hb + 1) * 512],
                        start=(i == 0), stop=(i == KI - 1))
                evict(ps, m, hb)
        if r == 1:
            next_pre = yield
    if len(_tiles(ncols)) < 2:
        next_pre = yield
    yield next_pre


def build(C, S, debug=False, use_silu=True):
    """Build the per-core Bass module. C: expert capacity, S: shared rows."""
    assert C % 256 == 0 and C >= RT and S % RT == 0
    nc = bacc.Bacc(None, target_bir_lowering=False, debug=debug)
    with tile.TileContext(nc) as tc:
        with tc.tile_pool(name="dram", bufs=1, space="DRAM") as dram:
            xtd = dram.tile((128, KH, C), F32R, kind="ExternalInput", name="xtd", uniquify=False)
            xts = dram.tile((128, KH, S), F32R, kind="ExternalInput", name="xts", uniquify=False)
            w1e = dram.tile((128, KH, I), F32R, kind="ExternalInput", name="w1e", uniquify=False)
            w2e = dram.tile((128, KI, H), F32R, kind="ExternalInput", name="w2e", uniquify=False)
            ws1 = dram.tile((128, KH, I), F32R, kind="ExternalInput", name="ws1", uniquify=False)
            ws2 = dram.tile((128, KI, H), F32R, kind="ExternalInput", name="ws2", uniquify=False)
            gate = dram.tile((128, C // 128), F32, kind="ExternalInput", name="gate", uniquify=False)
            yd = dram.tile((128, C // 128, H), F32, kind="ExternalOutput", name="yd", uniquify=False)
            ys = dram.tile((128, S // 128, H), F32, kind="ExternalOutput", name="ys", uniquify=False)
            htd = dram.tile((128, KI, C), F32R, name="htd", uniquify=False)
            hts = dram.tile((128, KI, S), F32R, name="hts", uniquify=False)

            with (
                tc.tile_pool(name="wpool", bufs=1) as wpool,
                tc.tile_pool(name="wpre", bufs=NPRE) as wpre_pool,
                tc.tile_pool(name="inpool", bufs=IN_BUFS) as in_pool,
                tc.tile_pool(name="outpool", bufs=6) as out_pool,
                tc.tile_pool(name="psum", bufs=8, space="PSUM") as psum_pool,
                tc.tile_pool(name="const", bufs=1) as const_pool,
            ):
                pools = (in_pool, out_pool, psum_pool)
                scale_sb = const_pool.tile([128, C // 128], F32, name="scale_sb")
                nc.sync.dma_start(scale_sb[:], gate[:])

                def load_pre(dram_w, tag_id):
                    pre = []
                    for k in range(NPRE):
                        t = wpre_pool.tile([128, H], F32R, tag="wpre",
                                           name=f"wpre_{tag_id}_{k}")
                        tv = t[:, :dram_w.shape[2]]
                        nc.sync.dma_start(tv, dram_w[:, k, :])
                        pre.append(tv)
                    return pre

                gen = _emit_pass1(nc, pools, wpool, w1e, None, xtd, htd, C, use_silu, "a")
                next(gen)
                pre_b = gen.send(load_pre(ws1, "b"))

                gen = _emit_pass1(nc, pools, wpool, ws1, pre_b, xts, hts, S, use_silu, "b")
                next(gen)
                pre_c = gen.send(load_pre(w2e, "c"))

                gen = _emit_pass2(nc, pools, wpool, w2e, pre_c, htd, yd, C, scale_sb, "c")
                next(gen)
                pre_d = gen.send(load_pre(ws2, "d"))

                gen = _emit_pass2(nc, pools, wpool, ws2, pre_d, hts, ys, S, None, "d")
                next(gen)
                try:
                    gen.send(None)
                except StopIteration:
                    pass

    nc.compile()
    return nc


def _get_built(C, S):
    key = (C, S)
    if key not in _BUILD_CACHE:
        _BUILD_CACHE[key] = build(C, S)
    return _BUILD_CACHE[key]


def _to_kxm_layout(a):
    """[K, M] -> [128, K/128, M] with logical row k at (k%128, k//128)."""
    k, m_ = a.shape
    return np.ascontiguousarray(a.reshape(k // 128, 128, m_).transpose(1, 0, 2))


def _round_fp32r(a):
    """Round fp32 to the fp32r grid (RNE to 1s+8e+11m; low 12 bits zero)."""
    u = np.ascontiguousarray(a).view(np.uint32)
    lsb = (u >> 12) & 1
    return ((u + 0x7FF + lsb) & 0xFFFFF000).view(np.float32)


def route_and_dispatch(xf, w_router):
    """Host router: returns (sorted token ids, gates, per-expert offsets, capacity)."""
    T = xf.shape[0]
    logits = xf @ w_router                       # [T, E]
    order = np.argsort(-logits, axis=1, kind="stable")[:, :2]
    mx = logits.max(axis=1, keepdims=True)
    p = np.exp(logits - mx)
    p /= p.sum(axis=1, keepdims=True)
    tk = np.take_along_axis(p, order, axis=1)    # [T, 2]
    g = tk / tk.sum(axis=1, keepdims=True)

    pe = order.ravel()                           # expert id per (token, slot) pair
    ptok = np.repeat(np.arange(T, dtype=np.int64), 2)
    pg = g.astype(np.float32).ravel()
    perm = np.argsort(pe, kind="stable")
    stok, sg = ptok[perm], pg[perm]
    counts = np.bincount(pe, minlength=E)
    offs = np.zeros(E + 1, dtype=np.int64)
    np.cumsum(counts, out=offs[1:])
    C = max(512, int(-(-counts.max() // 256) * 256))
    return stok, sg, offs, C


def prepare(x, w_shared1, w_shared2, w1, w2, w_router):
    """Host-side routing + dispatch. Returns (in_maps, meta)."""
    x = np.asarray(x, dtype=np.float32)
    w_shared1 = np.asarray(w_shared1, dtype=np.float32)
    w_shared2 = np.asarray(w_shared2, dtype=np.float32)
    w1 = np.asarray(w1, dtype=np.float32)
    w2 = np.asarray(w2, dtype=np.float32)
    w_router = np.asarray(w_router, dtype=np.float32)

    B, Sq, _ = x.shape
    T = B * Sq
    S = T // NCORES                              # shared-expert rows per core
    xf = x.reshape(T, H)

    stok, sg, offs, C = route_and_dispatch(xf, w_router)

    # pre-round matmul operands to the fp32r grid (router used unrounded xf)
    xf = _round_fp32r(xf)
    ws1_l = _to_kxm_layout(_round_fp32r(w_shared1))
    ws2_l = _to_kxm_layout(_round_fp32r(w_shared2))
    w1 = _round_fp32r(w1)
    w2 = _round_fp32r(w2)

    in_maps = []
    for e in range(NCORES):
        toks = stok[offs[e]:offs[e + 1]]
        n = len(toks)
        xd = np.zeros((C, H), np.float32)
        xd[:n] = xf[toks]
        gate_v = np.zeros(C, np.float32)
        gate_v[:n] = sg[offs[e]:offs[e + 1]]
        xs = xf[e * S:(e + 1) * S]
        in_maps.append({
            "xtd": np.ascontiguousarray(xd.reshape(C, KH, 128).transpose(2, 1, 0)),
            "xts": np.ascontiguousarray(xs.reshape(S, KH, 128).transpose(2, 1, 0)),
            "w1e": _to_kxm_layout(w1[e]),
            "w2e": _to_kxm_layout(w2[e]),
            "ws1": ws1_l,
            "ws2": ws2_l,
            "gate": np.ascontiguousarray(gate_v.reshape(C // 128, 128).T),
        })

    meta = (B, Sq, T, S, C, stok, offs)
    return in_maps, meta


def combine(results, meta):
    """Host-side gather/unshard of per-core outputs to the full output."""
    B, Sq, T, S, C, stok, offs = meta
    out = np.zeros((T, H), np.float32)
    for e in range(NCORES):
        toks = stok[offs[e]:offs[e + 1]]
        ydp = results[e]["yd"].transpose(1, 0, 2).reshape(C, H)
        out[toks] += ydp[:len(toks)]
        ysp = results[e]["ys"].transpose(1, 0, 2).reshape(S, H)
        out[e * S:(e + 1) * S] += ysp
    return out.reshape(B, Sq, H)


def kernel(x, w_shared1, w_shared2, w1, w2, w_router):
    in_maps, meta = prepare(x, w_shared1, w_shared2, w1, w2, w_router)
    C, S = meta[4], meta[3]
    nc = _get_built(C, S)
    res = run_bass_kernel_spmd(nc, in_maps, core_ids=list(range(NCORES)))
    return combine(res.results, meta)



# revision 16
# speedup vs baseline: 164.4307x; 164.4307x over previous
"""DeepSeekMoE layer on 8 TRN2 NeuronCores — expert-parallel with host token dispatch.

Reference computation (per token):
    shared = silu(x @ ws1) @ ws2
    router: softmax(x @ w_router) -> top-2 -> renormalize -> gates
    routed = sum_{e in top2} gate_e * silu(x @ w1[e]) @ w2[e]
    out    = shared + routed

Sharding: expert-parallel. Core e receives the (padded) bucket of all token
rows routed to expert e (capacity C), plus a 1/8 slice of all tokens for the
shared expert. Routing (softmax/top-k) and the dispatch/combine permutations
run on the host; all GEMMs + SiLU + gate scaling run on device.

Per-core device kernel (two passes through a DRAM-resident intermediate):
    pass1: hT = silu(w1e.T @ xT)   [I x R] with I=1408 on partitions
    pass2: y  = hT.T @ w2e         [R x H] scaled per-row by the gate
All matmuls run as float32r (full fp32 storage; TensorE full-rate mode).
"""

import numpy as np

import concourse.bass as bass  # noqa: F401  (bass types via bacc/tile)
import concourse.mybir as mybir
import concourse.tile as tile
from concourse import bacc
from concourse.bass_utils import run_bass_kernel_spmd

H = 2048          # hidden
I = 1408          # moe intermediate
E = 8             # routed experts == n cores
NCORES = 8
RT = 512          # token tile (columns of xT / rows of y) per step
KH = H // 128     # 16 k-tiles over hidden
KI = I // 128     # 11 k-tiles over intermediate
F32 = mybir.dt.float32
F32R = mybir.dt.float32r

_BUILD_CACHE: dict = {}


def _r(ap):
    return ap


NPRE = 4          # weight k-slices prefetched into wpre for the next phase
STORE_ENGINE = "sync"   # which engine issues output stores: sync|gpsimd|scalar|vector
IN_BUFS = 4


def _store(nc, dst, src):
    getattr(nc, STORE_ENGINE).dma_start(dst, src)


def _silu_evict(nc, out_pool, ps, tag_id, use_silu, sz=RT):
    ev = out_pool.tile([128, RT], F32R, tag="ev", name=f"ev_{tag_id}")
    if use_silu:
        nc.scalar.activation(ev[:, :sz], ps[:], mybir.ActivationFunctionType.Silu)
    else:
        sg = out_pool.tile([128, RT], F32, tag="sg", bufs=2, name=f"sg_{tag_id}")
        nc.scalar.activation(sg[:, :sz], ps[:], mybir.ActivationFunctionType.Sigmoid)
        nc.vector.tensor_mul(ev[:, :sz], ps[:], sg[:, :sz])
    return ev


def _tiles(ncols):
    """R-tile (offset, size) list: 512-tiles plus an optional 256 tail."""
    out, off = [], 0
    while ncols - off >= RT:
        out.append((off, RT))
        off += RT
    if ncols - off:
        assert (ncols - off) % 256 == 0
        out.append((off, ncols - off))
        off = ncols
    return out


def _emit_pass1(nc, pools, wpool, w_dram, w_pre, xt, ht, ncols, use_silu, ph):
    """ht[:, i, :] = silu(w.T @ xT) — I on partitions, tokens on free.

    First R-tile: per-k sliced stream+weight DMAs in consumption order and a
    k-outer chunked matmul loop, so the PE consumes slices as they arrive.
    Later R-tiles: i-outer / k-inner with whole-half stream DMAs.
    """
    in_pool, out_pool, psum_pool = pools
    npre = len(w_pre) if w_pre else 0

    # --- first R-tile: sliced, streaming ---
    cs = slice(0, RT)
    xh0 = in_pool.tile([128, 8, RT], F32R, tag="sin", name=f"xh0_{ph}_0")
    xh1 = in_pool.tile([128, 8, RT], F32R, tag="sin", name=f"xh1_{ph}_0")
    w = wpool.tile([128, KH, I], F32R, tag="w", name=f"w_{ph}")

    def xslice(k):
        return (xh0 if k < 8 else xh1)[:, k % 8, :]

    if npre:
        # stream slices first (prefetchable), then the WAR-blocked weight rest
        for k in range(KH):
            nc.sync.dma_start(xslice(k), xt[:, k, cs])
        for k in range(npre, KH):
            nc.sync.dma_start(w[:, k, :], w_dram[:, k, :])
    else:
        # cold start: interleave in consumption order
        for k in range(KH):
            nc.sync.dma_start(xslice(k), xt[:, k, cs])
            nc.sync.dma_start(w[:, k, :], w_dram[:, k, :])

    for lo, hi in ((0, 6), (6, KI)):
        pss = [psum_pool.tile([128, RT], F32, tag="ps", name=f"ps_{ph}_0_{i}")
               for i in range(lo, hi)]
        for k in range(KH):
            w_k = w_pre[k] if k < npre else w[:, k, :]
            for i in range(lo, hi):
                nc.tensor.matmul(
                    pss[i - lo][:], w_k[:, i * 128:(i + 1) * 128], xslice(k),
                    start=(k == 0), stop=(k == KH - 1))
        for i in range(lo, hi):
            ev = _silu_evict(nc, out_pool, pss[i - lo], f"{ph}_0_{i}", use_silu)
            _store(nc, ht[:, i, cs], ev[:])
    if npre:
        # also load the wpre-covered slices into the main tile for later R-tiles
        for k in range(npre):
            nc.sync.dma_start(w[:, k, :], w_dram[:, k, :])

    next_pre = None

    # --- remaining R-tiles (maybe a 256-wide tail) ---
    for r, (off, sz) in enumerate(_tiles(ncols)):
        if r == 0:
            continue
        cs = slice(off, off + sz)
        xh0 = in_pool.tile([128, 8, sz], F32R, tag="sin", name=f"xh0_{ph}_{r}")
        xh1 = in_pool.tile([128, 8, sz], F32R, tag="sin", name=f"xh1_{ph}_{r}")
        nc.sync.dma_start(xh0[:], xt[:, 0:8, cs])
        nc.sync.dma_start(xh1[:], xt[:, 8:16, cs])
        for i in range(KI):
            ps = psum_pool.tile([128, sz], F32, tag="ps", name=f"ps_{ph}_{r}_{i}")
            for k in range(KH):
                nc.tensor.matmul(
                    ps[:], w[:, k, i * 128:(i + 1) * 128],
                    (xh0 if k < 8 else xh1)[:, k % 8, :],
                    start=(k == 0), stop=(k == KH - 1))
            ev = _silu_evict(nc, out_pool, ps, f"{ph}_{r}_{i}", use_silu, sz)
            _store(nc, ht[:, i, cs], ev[:, :sz])
        if r == 1:
            next_pre = yield  # build() emits the next phase's wpre DMAs here
    if len(_tiles(ncols)) < 2:
        next_pre = yield
    yield next_pre


def _emit_pass2(nc, pools, wpool, w_dram, w_pre, ht, y, ncols, scale_sb, ph):
    """y[m, :] = (hT.T @ w2) * gate[m] — tokens on partitions.

    First R-tile: per-i sliced DMAs + i-outer over two chunks of 8 psum banks.
    Later R-tiles: i-inner.
    """
    in_pool, out_pool, psum_pool = pools
    npre = len(w_pre) if w_pre else 0

    def evict(ps, m, hblk):
        yt = out_pool.tile([128, 512], F32, tag="ev", name=f"y_{ph}_{m}_{hblk}")
        if scale_sb is not None:
            nc.vector.tensor_scalar_mul(yt[:], ps[:], scale_sb[:, m:m + 1])
        else:
            nc.vector.tensor_copy(yt[:], ps[:])
        _store(nc, y[:, m, hblk * 512:(hblk + 1) * 512], yt[:])

    # --- first R-tile: sliced, streaming ---
    cs = slice(0, RT)
    hh0 = in_pool.tile([128, 6, RT], F32R, tag="sin", name=f"hh0_{ph}_0")
    hh1 = in_pool.tile([128, 5, RT], F32R, tag="sin", name=f"hh1_{ph}_0")
    w = wpool.tile([128, KI, H], F32R, tag="w", name=f"w_{ph}")

    def hslice(i):
        return (hh0 if i < 6 else hh1)[:, i if i < 6 else i - 6, :]

    def lhs(i, c):
        return hslice(i)[:, c * 128:(c + 1) * 128]

    for i in range(KI):
        nc.sync.dma_start(hslice(i), ht[:, i, cs])
    for i in range(npre, KI):
        nc.sync.dma_start(w[:, i, :], w_dram[:, i, :])

    pairs = [(c, hb) for c in range(RT // 128) for hb in range(H // 512)]
    for chunk in (pairs[:8], pairs[8:]):
        pss = {p: psum_pool.tile([128, 512], F32, tag="ps",
                                 name=f"ps_{ph}_0_{p[0]}_{p[1]}")
               for p in chunk}
        for i in range(KI):
            w_i = w_pre[i] if i < npre else w[:, i, :]
            for (c, hb) in chunk:
                nc.tensor.matmul(
                    pss[(c, hb)][:], lhs(i, c), w_i[:, hb * 512:(hb + 1) * 512],
                    start=(i == 0), stop=(i == KI - 1))
        for (c, hb) in chunk:
            evict(pss[(c, hb)], c, hb)
    for i in range(npre):
        nc.sync.dma_start(w[:, i, :], w_dram[:, i, :])

    next_pre = None

    # --- remaining R-tiles (maybe a 256-wide tail) ---
    for r, (off, sz) in enumerate(_tiles(ncols)):
        if r == 0:
            continue
        cs = slice(off, off + sz)
        hh0 = in_pool.tile([128, 6, sz], F32R, tag="sin", name=f"hh0_{ph}_{r}")
        hh1 = in_pool.tile([128, 5, sz], F32R, tag="sin", name=f"hh1_{ph}_{r}")
        nc.sync.dma_start(hh0[:], ht[:, 0:6, cs])
        nc.sync.dma_start(hh1[:], ht[:, 6:KI, cs])
        for c in range(sz // 128):
            for hb in range(H // 512):
                m = off // 128 + c
                ps = psum_pool.tile([128, 512], F32, tag="ps", name=f"ps_{ph}_{m}_{hb}")
                for i in range(KI):
                    src = hh0 if i < 6 else hh1
                    j = i if i < 6 else i - 6
                    nc.tensor.matmul(
                        ps[:], src[:, j, c * 128:(c + 1) * 128],
                        w[:, i, hb * 512:(hb + 1) * 512],
                        start=(i == 0), stop=(i == KI - 1))
                evict(ps, m, hb)
        if r == 1:
            next_pre = yield
    if len(_tiles(ncols)) < 2:
        next_pre = yield
    yield next_pre


def build(C, S, debug=False, use_silu=True, reps=1):
    """Build the per-core Bass module. C: expert capacity, S: shared rows.

    reps>1 repeats the whole computation in one NEFF (timing use only)."""
    assert C % 256 == 0 and C >= RT and S % RT == 0
    nc = bacc.Bacc(None, target_bir_lowering=False, debug=debug)
    with tile.TileContext(nc) as tc:
        with tc.tile_pool(name="dram", bufs=1, space="DRAM") as dram:
            xtd = dram.tile((128, KH, C), F32R, kind="ExternalInput", name="xtd", uniquify=False)
            xts = dram.tile((128, KH, S), F32R, kind="ExternalInput", name="xts", uniquify=False)
            w1e = dram.tile((128, KH, I), F32R, kind="ExternalInput", name="w1e", uniquify=False)
            w2e = dram.tile((128, KI, H), F32R, kind="ExternalInput", name="w2e", uniquify=False)
            ws1 = dram.tile((128, KH, I), F32R, kind="ExternalInput", name="ws1", uniquify=False)
            ws2 = dram.tile((128, KI, H), F32R, kind="ExternalInput", name="ws2", uniquify=False)
            gate = dram.tile((128, C // 128), F32, kind="ExternalInput", name="gate", uniquify=False)
            yd = dram.tile((128, C // 128, H), F32, kind="ExternalOutput", name="yd", uniquify=False)
            ys = dram.tile((128, S // 128, H), F32, kind="ExternalOutput", name="ys", uniquify=False)
            htd = dram.tile((128, KI, C), F32R, name="htd", uniquify=False)
            hts = dram.tile((128, KI, S), F32R, name="hts", uniquify=False)

            with (
                tc.tile_pool(name="wpool", bufs=1) as wpool,
                tc.tile_pool(name="wpre", bufs=NPRE) as wpre_pool,
                tc.tile_pool(name="inpool", bufs=IN_BUFS) as in_pool,
                tc.tile_pool(name="outpool", bufs=6) as out_pool,
                tc.tile_pool(name="psum", bufs=8, space="PSUM") as psum_pool,
                tc.tile_pool(name="const", bufs=1) as const_pool,
            ):
                pools = (in_pool, out_pool, psum_pool)
                scale_sb = const_pool.tile([128, C // 128], F32, name="scale_sb")
                nc.sync.dma_start(scale_sb[:], gate[:])

                def load_pre(dram_w, tag_id):
                    pre = []
                    for k in range(NPRE):
                        t = wpre_pool.tile([128, H], F32R, tag="wpre",
                                           name=f"wpre_{tag_id}_{k}")
                        tv = t[:, :dram_w.shape[2]]
                        nc.sync.dma_start(tv, dram_w[:, k, :])
                        pre.append(tv)
                    return pre

                pre_a = None
                for rep in range(reps):
                    gen = _emit_pass1(nc, pools, wpool, w1e, pre_a, xtd, htd, C,
                                      use_silu, f"a{rep}")
                    next(gen)
                    pre_b = gen.send(load_pre(ws1, f"b{rep}"))

                    gen = _emit_pass1(nc, pools, wpool, ws1, pre_b, xts, hts, S,
                                      use_silu, f"b{rep}")
                    next(gen)
                    pre_c = gen.send(load_pre(w2e, f"c{rep}"))

                    gen = _emit_pass2(nc, pools, wpool, w2e, pre_c, htd, yd, C,
                                      scale_sb, f"c{rep}")
                    next(gen)
                    pre_d = gen.send(load_pre(ws2, f"d{rep}"))

                    gen = _emit_pass2(nc, pools, wpool, ws2, pre_d, hts, ys, S,
                                      None, f"d{rep}")
                    next(gen)
                    pre_a = gen.send(load_pre(w1e, f"a{rep + 1}")
                                     if rep + 1 < reps else None)

    nc.compile()
    return nc


def _get_built(C, S):
    key = (C, S)
    if key not in _BUILD_CACHE:
        _BUILD_CACHE[key] = build(C, S)
    return _BUILD_CACHE[key]


def _to_kxm_layout(a):
    """[K, M] -> [128, K/128, M] with logical row k at (k%128, k//128)."""
    k, m_ = a.shape
    return np.ascontiguousarray(a.reshape(k // 128, 128, m_).transpose(1, 0, 2))


def _round_fp32r(a):
    """Round fp32 to the fp32r grid (RNE to 1s+8e+11m; low 12 bits zero)."""
    u = np.ascontiguousarray(a).view(np.uint32)
    lsb = (u >> 12) & 1
    return ((u + 0x7FF + lsb) & 0xFFFFF000).view(np.float32)


def route_and_dispatch(xf, w_router):
    """Host router: returns (sorted token ids, gates, per-expert offsets, capacity)."""
    T = xf.shape[0]
    logits = xf @ w_router                       # [T, E]
    order = np.argsort(-logits, axis=1, kind="stable")[:, :2]
    mx = logits.max(axis=1, keepdims=True)
    p = np.exp(logits - mx)
    p /= p.sum(axis=1, keepdims=True)
    tk = np.take_along_axis(p, order, axis=1)    # [T, 2]
    g = tk / tk.sum(axis=1, keepdims=True)

    pe = order.ravel()                           # expert id per (token, slot) pair
    ptok = np.repeat(np.arange(T, dtype=np.int64), 2)
    pg = g.astype(np.float32).ravel()
    perm = np.argsort(pe, kind="stable")
    stok, sg = ptok[perm], pg[perm]
    counts = np.bincount(pe, minlength=E)
    offs = np.zeros(E + 1, dtype=np.int64)
    np.cumsum(counts, out=offs[1:])
    C = max(512, int(-(-counts.max() // 256) * 256))
    return stok, sg, offs, C


def prepare(x, w_shared1, w_shared2, w1, w2, w_router):
    """Host-side routing + dispatch. Returns (in_maps, meta)."""
    x = np.asarray(x, dtype=np.float32)
    w_shared1 = np.asarray(w_shared1, dtype=np.float32)
    w_shared2 = np.asarray(w_shared2, dtype=np.float32)
    w1 = np.asarray(w1, dtype=np.float32)
    w2 = np.asarray(w2, dtype=np.float32)
    w_router = np.asarray(w_router, dtype=np.float32)

    B, Sq, _ = x.shape
    T = B * Sq
    S = T // NCORES                              # shared-expert rows per core
    xf = x.reshape(T, H)

    stok, sg, offs, C = route_and_dispatch(xf, w_router)

    # pre-round matmul operands to the fp32r grid (router used unrounded xf)
    xf = _round_fp32r(xf)
    ws1_l = _to_kxm_layout(_round_fp32r(w_shared1))
    ws2_l = _to_kxm_layout(_round_fp32r(w_shared2))
    w1 = _round_fp32r(w1)
    w2 = _round_fp32r(w2)

    in_maps = []
    for e in range(NCORES):
        toks = stok[offs[e]:offs[e + 1]]
        n = len(toks)
        xd = np.zeros((C, H), np.float32)
        xd[:n] = xf[toks]
        gate_v = np.zeros(C, np.float32)
        gate_v[:n] = sg[offs[e]:offs[e + 1]]
        xs = xf[e * S:(e + 1) * S]
        in_maps.append({
            "xtd": np.ascontiguousarray(xd.reshape(C, KH, 128).transpose(2, 1, 0)),
            "xts": np.ascontiguousarray(xs.reshape(S, KH, 128).transpose(2, 1, 0)),
            "w1e": _to_kxm_layout(w1[e]),
            "w2e": _to_kxm_layout(w2[e]),
            "ws1": ws1_l,
            "ws2": ws2_l,
            "gate": np.ascontiguousarray(gate_v.reshape(C // 128, 128).T),
        })

    meta = (B, Sq, T, S, C, stok, offs)
    return in_maps, meta


def combine(results, meta):
    """Host-side gather/unshard of per-core outputs to the full output."""
    B, Sq, T, S, C, stok, offs = meta
    out = np.zeros((T, H), np.float32)
    for e in range(NCORES):
        toks = stok[offs[e]:offs[e + 1]]
        ydp = results[e]["yd"].transpose(1, 0, 2).reshape(C, H)
        out[toks] += ydp[:len(toks)]
        ysp = results[e]["ys"].transpose(1, 0, 2).reshape(S, H)
        out[e * S:(e + 1) * S] += ysp
    return out.reshape(B, Sq, H)


def kernel(x, w_shared1, w_shared2, w1, w2, w_router):
    in_maps, meta = prepare(x, w_shared1, w_shared2, w1, w2, w_router)
    C, S = meta[4], meta[3]
    nc = _get_built(C, S)
    res = run_bass_kernel_spmd(nc, in_maps, core_ids=list(range(NCORES)))
    return combine(res.results, meta)

